# revision 2
# baseline (speedup 1.0000x reference)
"""Trainium2 Bass kernel for nn_AttentiveTransformer (fc -> GhostBN -> *prior -> sparsemax).

308us total (steady 301us + 7us startup) vs 334us predecessor; the
sustained-throttle (P0 ~1.95GHz) pure-MM stream floor is ~269us steady.

Schedule (per core: 2048 batch rows, 8 segments of 256):
- bf16 W/x matmul, h^T[m] accumulated k-major in PSUM pair-banks
  ([128, 2x256] = 1 full bank holds 2 m-tiles; start=True only on the
  bank's FIRST MM -- the start flag clears the whole bank's has_written).
- Evac per pair-bank: 4x bn_stats (walrus only allows single-group) + ONE
  fat [128,512] ACT copy to f16 (2.6x cheaper than 4 small copies).
- GhostBN scale/shift from bn_stats halves; 32-op mixed ACT/DVE apply
  (faster than a broadcast-AP whole-tile DVE op).
- Blocked XBAR transposes (both on the SYNC queue: the XBAR is shared and
  not re-entrant -- two queues corrupt; on sync they never delay ACT evac)
  then prior-multiply with strided DVE reads, top-8-per-quarter compaction,
  5 Newton iterations for sparsemax tau (two row-chains interleaved),
  relu + store.
- Segment 0 runs k-major across ALL 16 m-tiles (the whole PSUM), so each
  arriving WT[k] unlocks 16 MMs: the cold 8.4MB W load (~20us, HBM-bound)
  hides behind seg-0 compute; cold x slab goes first on the gpsimd queue.
- Tails software-pipelined one segment behind their matmuls.

Self-contained: hardcodes shapes B=16384, D=2048, 8-core data-parallel split
over the batch dim. kernel(**inputs) takes full inputs, returns full output.
"""
import numpy as np
import ml_dtypes
from contextlib import ExitStack

import concourse.bacc as bacc
import concourse.tile as tile
import concourse.mybir as mybir
from concourse.bass_utils import run_bass_kernel_spmd

f32 = mybir.dt.float32
bf16 = mybir.dt.bfloat16
f16 = mybir.dt.float16
AF = mybir.ActivationFunctionType
ALU = mybir.AluOpType
AX = mybir.AxisListType

N_CORES = 8
B_FULL = 16384
D = 2048                  # D_in == D_out == 2048
BL = B_FULL // N_CORES    # 2048 rows per core
P = 128
KT = D // P               # 16 k-tiles (contraction)
MT = D // P               # 16 m-tiles (output d)
SEG = 256                 # batch rows per segment (2 tiles of 128)
NSEG = BL // SEG          # 8
NIT = 5                   # Newton iterations on the compacted candidates
EPS = 1e-5


def _body(nc, tc, ctx, X, PRI, Wd, Gd, Bd, OUT, repeat=1, cold=False, DBG=None):
    sb_const = ctx.enter_context(tc.tile_pool(name="const", bufs=1))
    wt_pool = ctx.enter_context(tc.tile_pool(name="wt", bufs=1))
    xt_pool = ctx.enter_context(tc.tile_pool(name="xt", bufs=2))
    hn_pool = ctx.enter_context(tc.tile_pool(name="hn", bufs=2))
    z_pool = ctx.enter_context(tc.tile_pool(name="z", bufs=2))
    prior_pool = ctx.enter_context(tc.tile_pool(name="prior", bufs=2))
    small_pool = ctx.enter_context(tc.tile_pool(name="small", bufs=2))
    stat_pool = ctx.enter_context(tc.tile_pool(name="stat", bufs=1))
    b6_pool = ctx.enter_context(tc.tile_pool(name="b6", bufs=2))
    mm_ps = ctx.enter_context(tc.tile_pool(name="mm_ps", bufs=8, space="PSUM"))
    zt_pool = ctx.enter_context(tc.tile_pool(name="zt", bufs=2))

    # --- constants ---
    eps_t = sb_const.tile([P, 1], f32)
    nc.vector.memset(eps_t[:], EPS)
    # gamma/beta: [2048] -> [128, 16] (col m = slice 128m..128m+128), then
    # expanded to [128, 32] with each col duplicated per virtual batch (2/seg)
    gtmp = sb_const.tile([P, MT], f32)
    btmp = sb_const.tile([P, MT], f32)
    nc.sync.dma_start(gtmp[:], Gd.rearrange("(m p) -> p m", p=P))
    nc.sync.dma_start(btmp[:], Bd.rearrange("(m p) -> p m", p=P))
    gx = sb_const.tile([P, 2 * MT], f32)
    bx = sb_const.tile([P, 2 * MT], f32)
    nc.vector.tensor_copy(gx[:, 0:2 * MT:2], gtmp[:])
    nc.vector.tensor_copy(gx[:, 1:2 * MT:2], gtmp[:])
    nc.vector.tensor_copy(bx[:, 0:2 * MT:2], btmp[:])
    nc.vector.tensor_copy(bx[:, 1:2 * MT:2], btmp[:])

    # --- phase 0: WT[k][i_part, o_free] = W^T  (host passes W already
    # transposed, so this is a plain tiled load, split across both HWDGE
    # queues; seg-0's x tiles are issued first on the sync queue) ---
    if not cold:
        XTa0 = xt_pool.tile([P, KT * SEG], bf16, tag="xta", name="xta_pre")
        nc.sync.dma_start(XTa0[:], X[0])
    WT = [wt_pool.tile([P, D], bf16, tag=f"wt{k}", name=f"wt{k}")
          for k in range(KT)]
    if not cold:
        for k in range(KT):
            eng = nc.scalar if k % 2 == 0 else nc.sync
            eng.dma_start(WT[k][:], Wd[k * P:(k + 1) * P, :])

    # --- phase 1: segments of 256 batch rows ---
    if repeat > 1:
        rep_cm = tc.For_i(0, repeat, 1)
        rep_cm.__enter__()
    XTa0c = None
    if cold:
        # W load inside the loop: each iteration is a complete cold run,
        # so (cold slope - warm slope) measures the startup phase.
        # seg-0's x slab goes FIRST on the (otherwise idle) gpsimd queue
        # so the k-major seg-0 MMs can start as soon as WT[0] lands.
        XTa0c = xt_pool.tile([P, KT * SEG], bf16, tag="xta", name="xta_cold")
        nc.gpsimd.dma_start(XTa0c[:], X[0])
        for k in range(KT):
            eng = nc.scalar if k % 2 == 0 else nc.sync
            eng.dma_start(WT[k][:], Wd[k * P:(k + 1) * P, :])

    def emit_pair_evac(pj, pr, B6, Hs):
        # bn_stats per (m,v) block (walrus only accepts single-group
        # BNStats) + one fat ACT copy per PSUM pair-bank; all PSUM APs
        # kept 2D so the PE/ACT fast paths stay engaged
        for b in range(4):
            nc.vector.bn_stats(B6[:, 4 * pj + b, :],
                               pr[:, b * P:(b + 1) * P])
        nc.scalar.activation(Hs[:, 4 * pj:4 * (pj + 1), :], pr[:],
                             AF.Identity)

    def emit_mm_group(s, mg, XTs, B6, Hs):
        # one group = 2 PSUM pair-banks = m-tiles 4*mg .. 4*mg+3
        prs = [mm_ps.tile([P, 2 * SEG], f32, tag="mm",
                          name=f"mm{s}_{mg}_{j}") for j in range(2)]
        # k-major issue: PE consumes WT[k] as each tile arrives
        for k in range(KT):
            for j in range(2):
                for i in range(2):
                    m = 4 * mg + 2 * j + i
                    # start=True clears the WHOLE bank's has_written bits,
                    # so only the first MM touching the bank may set it;
                    # the i=1 chain's k=0 lands on freshly-cleared bits
                    # (has_written=0 => plain write)
                    nc.tensor.matmul(prs[j][:, i * SEG:(i + 1) * SEG],
                                     WT[k][:, m * P:(m + 1) * P],
                                     XTs[k][:],
                                     start=(k == 0 and i == 0),
                                     stop=(k == KT - 1))
        for j in range(2):
            emit_pair_evac(2 * mg + j, prs[j], B6, Hs)

    def emit_tail_a(row0, B6, SC, SH, msum, dm, vr, Hs, zs, Cs, pchs, zts):
        # batched scale/shift for all 32 (m,v) pairs:
        #   mean = (me+mo)/2 ; var128 = M2e + M2o + 32*(me-mo)^2
        #   rstd = 1/sqrt(var128/128 + eps); scale = gamma*rstd
        #   shift = beta - mean*scale
        me_ap = B6[:, :, 1]
        mo_ap = B6[:, :, 4]
        m2e_ap = B6[:, :, 2]
        m2o_ap = B6[:, :, 5]
        nc.vector.tensor_tensor(msum[:], me_ap, mo_ap, ALU.add)
        nc.vector.tensor_tensor(dm[:], me_ap, mo_ap, ALU.subtract)
        nc.vector.tensor_tensor(vr[:], m2e_ap, m2o_ap, ALU.add)
        nc.vector.tensor_tensor(dm[:], dm[:], dm[:], ALU.mult)        # dm^2
        nc.vector.scalar_tensor_tensor(vr[:], dm[:], 32.0, vr[:],
                                       ALU.mult, ALU.add)             # var128
        nc.scalar.activation(vr[:], vr[:], AF.Sqrt, bias=eps_t[:],
                             scale=1.0 / P)                           # std
        nc.vector.reciprocal(vr[:], vr[:])                            # rstd
        nc.vector.tensor_tensor(SC[:], vr[:], gx[:], ALU.mult)        # scale
        nc.vector.tensor_scalar(msum[:], msum[:], 0.5, None, ALU.mult)
        nc.vector.tensor_tensor(msum[:], msum[:], SC[:], ALU.mult)
        nc.vector.tensor_tensor(SH[:], bx[:], msum[:], ALU.subtract)  # shift
        if DBG and row0 == 0:
            nc.sync.dma_start(DBG["hpre"], Hs[:])
        if DBG and row0 == SEG:
            nc.sync.dma_start(DBG["hpre1"], Hs[:])
            nc.sync.dma_start(DBG["b6"], B6[:])
            nc.sync.dma_start(DBG["scsh"][:, 0:2 * MT], SC[:])
            nc.sync.dma_start(DBG["scsh"][:, 2 * MT:4 * MT], SH[:])
        # apply (in place, f16 -> f16): hn = h*scale + shift, split
        # between ACT and DVE (a broadcast-AP whole-tile op measured 2x
        # slower on DVE than this 32-op mixed split)
        for m in range(MT):
            for v in range(2):
                bk = 2 * m + v
                dst = Hs[:, bk, :]
                if m % 2 == 0:
                    nc.scalar.activation(dst, dst, AF.Identity,
                                         bias=SH[:, bk:bk + 1],
                                         scale=SC[:, bk:bk + 1])
                else:
                    nc.vector.tensor_scalar(dst, dst, SC[:, bk:bk + 1],
                                            SH[:, bk:bk + 1],
                                            ALU.mult, ALU.add)
        if DBG and row0 == 0:
            nc.sync.dma_start(DBG["hpost"], Hs[:])


    def emit_tail_b(row0, B6, SC, SH, msum, dm, vr, Hs, zs, Cs, pchs, zts):
        # blocked transposes on the HWDGE queues' XBAR units (~14 ns per
        # 32x32 tile): PE does no transposes. Hs blocks are (m,v)
        # interleaved, so transpose the two contiguous m-halves on two
        # queues and pick each vb's blocks with strided DVE reads below.
        zta = zt_pool.tile([P, MT, P], f16, tag="zta", name="zta")
        ztb = zt_pool.tile([P, MT, P], f16, tag="ztb", name="ztb")
        nc.scalar.dma_start_transpose(zta[:], Hs[:, 0:MT, :])
        nc.scalar.dma_start_transpose(ztb[:], Hs[:, MT:2 * MT, :])
        if DBG and row0 == 0:
            nc.sync.dma_start(DBG["zt"][0], zta[:])
            nc.sync.dma_start(DBG["zt"][1], ztb[:])
        for u in range(2):
            z = zs[u]
            for q in range(4):
                src = zta if q < 2 else ztb
                b0 = 8 * (q % 2) + u
                nc.vector.tensor_tensor(z[:, q], src[:, b0:b0 + 7:2, :],
                                        pchs[u][:, q], ALU.mult)
                nc.vector.max(Cs[u][:, 8 * q:8 * q + 8], z[:, q])
            if DBG and row0 == 0:
                nc.sync.dma_start(DBG["z"][u], z[:])
                nc.sync.dma_start(DBG["cs"][u], Cs[u][:])

        # sparsemax per row-tile: pure-DVE Newton chains for the two
        # row-tiles, INTERLEAVED so each chain's dependent-op latency is
        # hidden behind the other chain's independent ops
        st = []
        for u in range(2):
            it = small_pool.tile([P, 8], f32, tag="it", name=f"it_{u}")
            relu_s = small_pool.tile([P, 32], f32, tag="relu_s",
                                     name=f"relu_{u}")
            sign_s = small_pool.tile([P, 32], f32, tag="sign_s",
                                     name=f"sign_{u}")
            # tau0 = max over the four 8th-largest values
            nc.vector.tensor_reduce(it[:, 5:6], Cs[u][:, 7:32:8], axis=AX.X,
                                    op=ALU.max)
            st.append((it, relu_s, sign_s))
        for _ in range(NIT):
            for u in range(2):
                it, relu_s, sign_s = st[u]
                C = Cs[u]
                macc, kacc = it[:, 1:2], it[:, 2:3]
                krec, delta = it[:, 3:4], it[:, 4:5]
                tpos, tmp = it[:, 5:6], it[:, 6:7]
                # sum relu(C - tau) = sum max(C,tau) - 32*tau  (macc via
                # elementwise max + add-reduce accumulator, all on DVE)
                nc.vector.tensor_scalar(relu_s[:], C[:], tpos, None,
                                        ALU.max, ALU.add, accum_out=macc)
                nc.vector.tensor_scalar(sign_s[:], C[:], tpos, None,
                                        ALU.is_gt, ALU.add, accum_out=kacc)
                nc.vector.reciprocal(krec, kacc)
                # tmp = macc - 32*tau = sum relu(C-tau)
                nc.vector.scalar_tensor_tensor(tmp, tpos, -32.0, macc,
                                               ALU.mult, ALU.add)
                # delta = (tmp - 1) * krec
                nc.vector.scalar_tensor_tensor(delta, tmp, -1.0, krec,
                                               ALU.add, ALU.mult)
                nc.vector.tensor_tensor(tpos, tpos, delta, ALU.add)
        for u in range(2):
            if DBG and row0 == 0:
                nc.sync.dma_start(DBG["tau"][u], st[u][0][:])
        for u in range(2):
            it = st[u][0]
            tneg, tpos = it[:, 0:1], it[:, 5:6]
            nc.vector.tensor_scalar(tneg, tpos, -1.0, None, ALU.mult)
            # out = relu(z + tneg), in place (f16), then DMA out
            z = zs[u]
            nc.vector.tensor_scalar(z[:], z[:], tneg, 0.0, ALU.add, ALU.max)
            nc.sync.dma_start(OUT[row0 + u * P: row0 + (u + 1) * P, :], z[:])

    prev = None
    for s in range(NSEG):
        row0 = s * SEG
        # x^T is pre-transposed on host: direct strided DMA into XTa
        if s == 0 and repeat == 1 and not cold:
            XTa = XTa0
        elif s == 0 and cold:
            XTa = XTa0c
        else:
            XTa = xt_pool.tile([P, KT * SEG], bf16, tag="xta", name=f"xta{s}")
            nc.sync.dma_start(XTa[:], X[s])
        XTs = [XTa[:, k * SEG:(k + 1) * SEG] for k in range(KT)]

        # matmul h^T[m] + ghost-BN stats (bn_stats per vb); H kept in SBUF f16
        B6 = b6_pool.tile([P, 2 * MT, 6], f32, tag="B6", name=f"B6_{s}")
        SC = stat_pool.tile([P, 2 * MT], f32, tag="SC")      # scale
        SH = stat_pool.tile([P, 2 * MT], f32, tag="SH")      # shift
        msum = stat_pool.tile([P, 2 * MT], f32, tag="msum")  # mean_e+mean_o
        dm = stat_pool.tile([P, 2 * MT], f32, tag="dm")
        vr = stat_pool.tile([P, 2 * MT], f32, tag="vr")
        Hs = hn_pool.tile([P, 2 * MT, P], f16, tag="h", name=f"h{s}")
        zs = [z_pool.tile([P, 4, 4, P], f16, tag="z", name=f"z{s}_{u}")
              for u in range(2)]
        Cs = [small_pool.tile([P, 32], f32, tag="C", name=f"C{s}_{u}")
              for u in range(2)]
        pchs = [prior_pool.tile([P, 4, 4, P], f16, tag="prior",
                                name=f"pch{s}_{u}")
                for u in range(2)]
        zts = (zt_pool.tile([P, MT, P], f16, tag="zta", name=f"zta{s}"),
               zt_pool.tile([P, MT, P], f16, tag="ztb", name=f"ztb{s}"))
        for u in range(2):
            nc.gpsimd.dma_start(pchs[u][:],
                                PRI[row0 + u * P: row0 + (u + 1) * P, :])
        cur = (row0, B6, SC, SH, msum, dm, vr, Hs, zs, Cs, pchs, zts)

        if s == 0:
            # seg 0: k-major across ALL 16 m-tiles (the full 8 PSUM banks),
            # so each arriving WT[k] enables 16 MMs (~1.7us of PE work vs
            # ~1.25us/tile W-DMA delivery) -- hides the cold W load behind
            # seg-0 compute instead of starving the PE k-tile by k-tile.
            pms0 = [mm_ps.tile([P, 2 * SEG], f32, tag="mm",
                               name=f"mm0_{j}") for j in range(8)]
            for k in range(KT):
                for j in range(8):
                    for i in range(2):
                        m = 2 * j + i
                        nc.tensor.matmul(pms0[j][:, i * SEG:(i + 1) * SEG],
                                         WT[k][:, m * P:(m + 1) * P],
                                         XTs[k][:],
                                         start=(k == 0 and i == 0),
                                         stop=(k == KT - 1))
            for j in range(8):
                emit_pair_evac(j, pms0[j], B6, Hs)
        else:
            # group 0 bridges the PE gap; stats+apply of seg s-1 overlap
            # groups 1-3; transposes+sparsemax of s-1 go after group 3
            emit_mm_group(s, 0, XTs, B6, Hs)
            if prev is not None:
                emit_tail_a(*prev)
            for mg in range(1, 4):
                emit_mm_group(s, mg, XTs, B6, Hs)
            if prev is not None:
                emit_tail_b(*prev)
        prev = cur
    emit_tail_a(*prev)
    emit_tail_b(*prev)
    if repeat > 1:
        rep_cm.__exit__(None, None, None)


def build(repeat=1, cold=False, dbg=False):
    nc = bacc.Bacc("TRN2", target_bir_lowering=False, debug=False)
    X = nc.dram_tensor("x", [NSEG, P, KT * SEG], bf16,
                   kind="ExternalInput").ap()
    PRI = nc.dram_tensor("prior", [BL, D], f16, kind="ExternalInput").ap()
    Wd = nc.dram_tensor("W", [D, D], bf16, kind="ExternalInput").ap()
    Gd = nc.dram_tensor("gamma", [D], f32, kind="ExternalInput").ap()
    Bd = nc.dram_tensor("beta", [D], f32, kind="ExternalInput").ap()
    OUT = nc.dram_tensor("out", [BL, D], f16, kind="ExternalOutput").ap()
    DBG = {}
    if dbg:
        DBG["hpre"] = nc.dram_tensor("dbg_hpre", [P, 2 * MT * P], f16,
                                     kind="ExternalOutput").ap()
        DBG["hpre1"] = nc.dram_tensor("dbg_hpre1", [P, 2 * MT * P], f16,
                                      kind="ExternalOutput").ap()
        DBG["hpost"] = nc.dram_tensor("dbg_hpost", [P, 2 * MT * P], f16,
                                      kind="ExternalOutput").ap()
        DBG["b6"] = nc.dram_tensor("dbg_b6", [P, 2 * MT * 6], f32,
                                   kind="ExternalOutput").ap()
        DBG["scsh"] = nc.dram_tensor("dbg_scsh", [P, 4 * MT], f16,
                                     kind="ExternalOutput").ap()
        DBG["zt"] = nc.dram_tensor("dbg_zt", [2, P, MT * P], f16,
                                   kind="ExternalOutput").ap()
        DBG["z"] = nc.dram_tensor("dbg_z", [2, P, D], f16,
                                  kind="ExternalOutput").ap()
        DBG["cs"] = nc.dram_tensor("dbg_cs", [2, P, 32], f32,
                                   kind="ExternalOutput").ap()
        DBG["tau"] = nc.dram_tensor("dbg_tau", [2, P, 8], f32,
                                    kind="ExternalOutput").ap()
    with tile.TileContext(nc) as tc, ExitStack() as ctx:
        _body(nc, tc, ctx, X, PRI, Wd, Gd, Bd, OUT, repeat=repeat,
              cold=cold, DBG=DBG)
    nc.compile()
    return nc


_NC = None


def make_in_maps(inputs):
    prior16 = np.asarray(inputs["prior"], dtype=np.float32).astype(
        ml_dtypes.float16 if hasattr(ml_dtypes, "float16") else np.float16)
    x = np.asarray(inputs["x"], dtype=np.float32).astype(ml_dtypes.bfloat16)
    # device kernel wants W^T [D_in, D_out]; transpose once on host
    W = np.ascontiguousarray(
        np.asarray(inputs["W"], dtype=np.float32).astype(ml_dtypes.bfloat16).T)
    gamma = np.ascontiguousarray(inputs["gamma"], dtype=np.float32)
    beta = np.ascontiguousarray(inputs["beta"], dtype=np.float32)
    in_maps = []
    for i in range(N_CORES):
        sl = slice(i * BL, (i + 1) * BL)
        xc = x[sl].reshape(NSEG, SEG, KT, P).transpose(0, 3, 2, 1)
        in_maps.append({"x": np.ascontiguousarray(xc).reshape(
                            NSEG, P, KT * SEG),
                        "prior": np.ascontiguousarray(prior16[sl]),
                        "W": W, "gamma": gamma, "beta": beta})
    return in_maps


def _run(inputs, trace=False, **kw):
    global _NC
    if _NC is None:
        _NC = build()
    in_maps = make_in_maps(inputs)
    res = run_bass_kernel_spmd(_NC, in_maps, list(range(N_CORES)),
                               trace=trace, **kw)
    out = np.concatenate([res.results[i]["out"] for i in range(N_CORES)],
                         axis=0).astype(np.float32)
    return out, res


def kernel(prior, x, W, gamma, beta):
    out, _ = _run({"prior": prior, "x": x, "W": W,
                   "gamma": gamma, "beta": beta})
    return out
